# revision 1
# baseline (speedup 1.0000x reference)
"""Trainium2 Bass kernel for causal local-window self-attention.

Model (matches the PyTorch/JAX reference):
    qkv = x @ w_attn;  q,k,v = split(qkv)
    per head: att = softmax(mask(q k^T / sqrt(hd)));  y = att @ v
    out = y @ w_proj

Shapes (hardcoded): B=2, T=2048, C=1024, H=16, hd=64, window=256.

Sharding: flatten (B,T) -> 4096 tokens, 8 chunks of 512 queries (one per
NeuronCore), each with a 256-token halo of keys/values.  Chunk-boundary
causality (incl. the batch boundary at token 2048) is handled by per-core
additive mask data, so all 8 cores run one identical SPMD program and the
host only slices / transposes / concatenates.

On-device dataflow (per core), all matmuls in float32r (full PE rate for
moving-dim >= 256, ~1.5e-4 matmul error):
  - q^T,k^T computed feature-major (w tile as lhsT, x^T as moving operand)
  - v computed token-major and packed into V_aug[k,65] with a ones column,
    so the attention AV matmul also produces softmax denominators
  - scores computed transposed s^T=[keys, q] in PSUM; band mask added on
    DVE; exp on ACT (no max subtraction: logits are O(5), fp32-safe)
  - denominators inverted on DVE, broadcast across partitions with a tiny
    selector matmul, applied during the PSUM->SBUF copy of y^T
  - out^T = w_proj^T @ y^T accumulated over feature chunks; host transposes
"""

import numpy as np

import concourse.bass as bass
import concourse.mybir as mybir
from concourse.tile import TileContext
from concourse.bass_utils import run_bass_kernel_spmd

F32 = mybir.dt.float32
F32R = mybir.dt.float32r

N_CORES = 8
B, T, C = 2, 2048, 1024
H, HD, W = 16, 64, 256
T_OWN = 512          # queries per core
HALO = 256
T_LOC = T_OWN + HALO  # keys/values per core
NEG = -1e9


# ---------------------------------------------------------------------------
# BIR post-pass: this walrus build only accepts one sync-wait per CTRL-class
# instruction; hoist extra waits onto NoOps inserted just before.
# ---------------------------------------------------------------------------
def _split_excess_waits(nc, max_waits=1):
    for fn in nc.m.functions:
        for blk in fn.blocks:
            insts = blk.instructions
            i = 0
            while i < len(insts):
                inst = insts[i]
                si = inst.sync_info
                if si is not None and si.on_wait and len(si.on_wait) > max_waits:
                    waits = list(si.on_wait)
                    keep = waits[-max_waits:]
                    extra = waits[:-max_waits]
                    nops = []
                    for j in range(0, len(extra), max_waits):
                        nop = mybir.InstNoOp(
                            name=nc.get_next_instruction_name(),
                            sync_info=mybir.SyncInfo(
                                on_wait=extra[j : j + max_waits], on_update=[]
                            ),
                            bass_nofuse=True,
                            engine=inst.engine,
                        )
                        nops.append(nop)
                    inst.sync_info = mybir.SyncInfo(
                        on_wait=keep, on_update=list(si.on_update)
                    )
                    for k, nop in enumerate(nops):
                        insts.insert(i + k, nop)
                        nc.register_instruction(nop)
                    i += len(nops)
                i += 1
    return nc


# ---------------------------------------------------------------------------
# Device program (identical on all 8 cores)
# ---------------------------------------------------------------------------
def build_nc(debug=False, reps=None):
    nc = bass.Bass()

    xT = nc.dram_tensor("xT", [C, T_LOC], F32R, kind="ExternalInput")
    wq = nc.dram_tensor("wq", [C, C], F32R, kind="ExternalInput")
    wk = nc.dram_tensor("wk", [C, C], F32R, kind="ExternalInput")
    wv = nc.dram_tensor("wv", [C, C], F32R, kind="ExternalInput")
    wp = nc.dram_tensor("wp", [C, C], F32R, kind="ExternalInput")
    maskb = nc.dram_tensor("maskb", [2, 4, 128, 256], F32, kind="ExternalInput")
    sel = nc.dram_tensor("sel", [16, C], F32R, kind="ExternalInput")
    outT = nc.dram_tensor("outT", [C, T_OWN], F32, kind="ExternalOutput")
    den_dram = nc.dram_tensor("den_dram", [16, T_OWN], F32)
    if debug:
        dbg_q = nc.dram_tensor("dbg_q", [128, 8, T_OWN], F32, kind="ExternalOutput")
        dbg_k = nc.dram_tensor("dbg_k", [128, 8, T_LOC], F32, kind="ExternalOutput")
        dbg_v = nc.dram_tensor("dbg_v", [128, 6, 16, 65], F32, kind="ExternalOutput")
        dbg_p = nc.dram_tensor("dbg_p", [128, 4, 256], F32, kind="ExternalOutput")
        dbg_r = nc.dram_tensor("dbg_r", [16, T_OWN], F32, kind="ExternalOutput")
        dbg_y = nc.dram_tensor("dbg_y", [128, 8, T_OWN], F32, kind="ExternalOutput")

    with TileContext(nc) as tc:
        with (
            tc.tile_pool(name="big", bufs=1) as big,
            tc.tile_pool(name="wtiles", bufs=2) as wtiles,
            tc.tile_pool(name="wvtiles", bufs=1) as wvtiles,
            tc.tile_pool(name="pt", bufs=2) as ptpool,
            tc.tile_pool(name="stage", bufs=2) as stage,
            tc.tile_pool(name="dbgp", bufs=1) as dbgp,
            tc.tile_pool(name="psq", bufs=2, space="PSUM") as psq,
            tc.tile_pool(name="pss", bufs=3, space="PSUM") as pss_pool,
            tc.tile_pool(name="psy", bufs=2, space="PSUM") as psy_pool,
        ):
          for _rep in range(reps or 1):
              # ---- resident inputs -------------------------------------------
              xts = big.tile([128, 8, T_LOC], F32R, tag="xts")
              nc.sync.dma_start(out=xts[:], in_=xT.rearrange("(o p) t -> p o t", p=128))
              mk = big.tile([128, 2, 4, 256], F32, tag="mk")
              nc.sync.dma_start(out=mk[:], in_=maskb.rearrange("a j p t -> p a j t"))
              sel_sb = big.tile([16, C], F32R, tag="sel")
              nc.sync.dma_start(out=sel_sb[:], in_=sel[:])

              ones_sb = big.tile([128, 1], F32, tag="ones")
              nc.vector.memset(ones_sb[:], 1.0)

              qTs = big.tile([128, 8, T_OWN], F32R, tag="qTs")
              kTs = big.tile([128, 8, T_LOC], F32R, tag="kTs")
              # V_aug: [part(keys%128), kc, head, 65]; col 64 of each head is 1.0
              vaug = big.tile([128, 6, 16, 65], F32R, tag="vaug")
              yTs = big.tile([128, 8, T_OWN], F32R, tag="yTs")
              recips = big.tile([16, T_OWN], F32, tag="recips")
              recips_r = big.tile([16, T_OWN], F32R, tag="recips_r")

              # ---- q^T, k^T (feature-major) ----------------------------------
              for oc in range(8):
                  wsl = wtiles.tile([128, 8, 128], F32R, tag="wsl")
                  nc.sync.dma_start(
                      out=wsl[:],
                      in_=wq[:, oc * 128 : (oc + 1) * 128].rearrange("(i p) m -> p i m", p=128),
                  )
                  ps = psq.tile([128, 512], F32, tag="ps_qkv")
                  for ic in range(8):
                      nc.tensor.matmul(
                          ps[:], wsl[:, ic], xts[:, ic, HALO:], start=(ic == 0), stop=(ic == 7)
                      )
                  nc.scalar.copy(out=qTs[:, oc], in_=ps[:])
              for oc in range(8):
                  wsl = wtiles.tile([128, 8, 128], F32R, tag="wsl")
                  nc.sync.dma_start(
                      out=wsl[:],
                      in_=wk[:, oc * 128 : (oc + 1) * 128].rearrange("(i p) m -> p i m", p=128),
                  )
                  for hf in range(2):
                      ps = psq.tile([128, 512], F32, tag="ps_qkv")
                      for ic in range(8):
                          nc.tensor.matmul(
                              ps[:, :384],
                              wsl[:, ic],
                              xts[:, ic, hf * 384 : (hf + 1) * 384],
                              start=(ic == 0),
                              stop=(ic == 7),
                          )
                      nc.scalar.copy(out=kTs[:, oc, hf * 384 : (hf + 1) * 384], in_=ps[:, :384])

              # ---- v (token-major) + ones column -----------------------------
              for h2 in range(2):
                  wvsl = wvtiles.tile([128, 8, 512], F32R, tag="wvsl")
                  nc.sync.dma_start(
                      out=wvsl[:],
                      in_=wv[:, h2 * 512 : (h2 + 1) * 512].rearrange("(i p) m -> p i m", p=128),
                  )
                  for kc in range(6):
                      ps = psq.tile([128, 512], F32, tag="ps_qkv")
                      for ic in range(8):
                          nc.tensor.matmul(
                              ps[:],
                              xts[:, ic, kc * 128 : (kc + 1) * 128],
                              wvsl[:, ic],
                              start=(ic == 0),
                              stop=(ic == 7),
                          )
                      # ps: [128 tokens, 512 vfeat] -> vaug[:, kc, h, 0:64]
                      nc.scalar.copy(
                          out=vaug[:, kc, h2 * 8 : (h2 + 1) * 8, 0:64],
                          in_=ps[:].rearrange("p (h d) -> p h d", d=64),
                      )
              for kc in range(6):
                  nc.vector.tensor_copy(
                      out=vaug[:, kc, :, 64:65],
                      in_=ones_sb[:, None, :].to_broadcast((128, 16, 1)),
                  )

              if debug:
                  for src, dst, shp in (
                      (qTs, dbg_q, [128, 8, T_OWN]),
                      (kTs, dbg_k, [128, 8, T_LOC]),
                      (vaug, dbg_v, [128, 6, 16, 65]),
                  ):
                      dtile = dbgp.tile(shp, F32, tag="dbg")
                      nc.vector.tensor_copy(out=dtile[:], in_=src[:])
                      nc.sync.dma_start(out=dst[:], in_=dtile[:])

              # ---- attention: per head, q-blocks of 256, key chunks of 128 ----
              for h in range(16):
                  pb = (h % 2) * 64  # partition base of this head's features
                  oc = h // 2
                  t = h // 2  # head-pair index for the recip broadcast
                  for qb in range(2):
                      ptile = ptpool.tile([128, 4, 256], F32R, tag="pt")
                      for j in range(4):
                          kc = qb * 2 + j  # key chunk [qb*256 + j*128, +128) local
                          ps = pss_pool.tile([128, 256], F32, tag="ps_s")
                          nc.tensor.matmul(
                              ps[:],
                              kTs[pb : pb + 64, oc, (qb * 2 + j) * 128 : (qb * 2 + j + 1) * 128],
                              qTs[pb : pb + 64, oc, qb * 256 : (qb + 1) * 256],
                              start=True,
                              stop=True,
                          )
                          nc.vector.tensor_add(out=ps[:], in0=ps[:], in1=mk[:, qb, j])
                          nc.scalar.activation(
                              out=ptile[:, j], in_=ps[:], func=mybir.ActivationFunctionType.Exp
                          )
                      if debug and h == 0 and qb == 0:
                          ptf = dbgp.tile([128, 4, 256], F32, tag="dbg")
                          nc.vector.tensor_copy(out=ptf[:], in_=ptile[:])
                          nc.sync.dma_start(out=dbg_p[:], in_=ptf[:])
                      ya = psy_pool.tile([128, 256], F32, tag="ps_y")
                      for j in range(4):
                          kc = qb * 2 + j
                          nc.tensor.matmul(
                              ya[:65],
                              vaug[:, kc, h],
                              ptile[:, j],
                              start=(j == 0),
                              stop=(j == 3),
                          )
                      # stash denominator row; normalize y^T after recip bcast
                      db = stage.tile([1, 256], F32, tag="den")
                      nc.vector.tensor_copy(out=db[:], in_=ya[64:65])
                      nc.sync.dma_start(
                          out=den_dram[h : h + 1, qb * 256 : (qb + 1) * 256],
                          in_=db[0:1, :],
                      )
                      # keep unnormalized y^T in SBUF for now
                      nc.vector.tensor_copy(
                          out=yTs[pb : pb + 64, oc, qb * 256 : (qb + 1) * 256], in_=ya[0:64]
                      )

              # ---- reciprocal + partition-broadcast + normalize --------------
              nc.sync.dma_start(out=recips[:], in_=den_dram[:])
              nc.vector.reciprocal(out=recips[:], in_=recips[:])
              nc.vector.tensor_copy(out=recips_r[:], in_=recips[:])
              for t in range(8):
                  rb = psq.tile([128, 512], F32, tag="ps_qkv")
                  nc.tensor.matmul(
                      rb[:], sel_sb[:, t * 128 : (t + 1) * 128], recips_r[:], start=True, stop=True
                  )
                  rb_sb = stage.tile([128, 512], F32, tag="rb_sb")
                  nc.scalar.copy(out=rb_sb[:], in_=rb[:])
                  for i in range(2):  # the two heads of the pair
                      h = 2 * t + i
                      pb = (h % 2) * 64
                      nc.vector.tensor_mul(
                          out=yTs[pb : pb + 64, t],
                          in0=yTs[pb : pb + 64, t],
                          in1=rb_sb[pb : pb + 64, :],
                      )

              if debug:
                  nc.sync.dma_start(out=dbg_r[:], in_=recips[:])
                  dy = dbgp.tile([128, 8, T_OWN], F32, tag="dbg")
                  nc.vector.tensor_copy(out=dy[:], in_=yTs[:])
                  nc.sync.dma_start(out=dbg_y[:], in_=dy[:])

              # ---- out projection: out^T = wp^T @ y^T ------------------------
              for oc in range(8):
                  wsl = wtiles.tile([128, 8, 128], F32R, tag="wsl")
                  nc.sync.dma_start(
                      out=wsl[:],
                      in_=wp[:, oc * 128 : (oc + 1) * 128].rearrange("(i p) m -> p i m", p=128),
                  )
                  ps = psq.tile([128, 512], F32, tag="ps_qkv")
                  for ic in range(8):
                      nc.tensor.matmul(
                          ps[:], wsl[:, ic], yTs[:, ic], start=(ic == 0), stop=(ic == 7)
                      )
                  ot = stage.tile([128, 512], F32, tag="ot")
                  nc.scalar.copy(out=ot[:], in_=ps[:])
                  nc.sync.dma_start(
                      out=outT.rearrange("(o p) t -> p o t", p=128)[:, oc], in_=ot[:]
                  )

    _split_excess_waits(nc)
    return nc


# ---------------------------------------------------------------------------
# Host-side sharding / unsharding
# ---------------------------------------------------------------------------
def _build_masks():
    """mask[qb, j, r, col]: 0 if query col of q-block qb may attend key
    qb*256+j*128+r (local coords), else NEG.  Variant 0: interior chunk;
    variant 1: first chunk of a batch (halo keys are invalid)."""
    r = np.arange(128)[:, None]
    col = np.arange(256)[None, :]
    masks = []
    for chunk0 in (False, True):
        m = np.full((2, 4, 128, 256), NEG, np.float32)
        for qb in range(2):
            for j in range(4):
                d = j * 128 + r  # key pos relative to q-block start
                valid = (col >= d - 256) & (col < d)
                if chunk0:
                    valid &= (d + qb * 256) >= 256
                m[qb, j][valid] = 0.0
        masks.append(m)
    return masks  # [interior, chunk0]


def _build_sel():
    """sel[h, t*128+m] = 1 iff head h supplies partition m of pair t's
    reciprocal broadcast (rows 0-63 <- even head, 64-127 <- odd head)."""
    s = np.zeros((16, C), np.float32)
    for t in range(8):
        s[2 * t, t * 128 : t * 128 + 64] = 1.0
        s[2 * t + 1, t * 128 + 64 : t * 128 + 128] = 1.0
    return s


def make_in_maps(x, w_attn, w_proj):
    xf = np.ascontiguousarray(x.reshape(B * T, C)).astype(np.float32)
    wq = np.ascontiguousarray(w_attn[:, :C]) * np.float32(1.0 / np.sqrt(HD))
    wk = np.ascontiguousarray(w_attn[:, C : 2 * C])
    wv = np.ascontiguousarray(w_attn[:, 2 * C :])
    wp = np.ascontiguousarray(w_proj).astype(np.float32)
    mask_int, mask_c0 = _build_masks()
    sel = _build_sel()

    in_maps = []
    for c in range(N_CORES):
        start = c * T_OWN
        xpad = np.zeros((T_LOC, C), np.float32)
        if c % 4 == 0:
            xpad[HALO:] = xf[start : start + T_OWN]
            m = mask_c0
        else:
            xpad[:] = xf[start - HALO : start + T_OWN]
            m = mask_int
        in_maps.append(
            {
                "xT": np.ascontiguousarray(xpad.T),
                "wq": wq,
                "wk": wk,
                "wv": wv,
                "wp": wp,
                "maskb": m,
                "sel": sel,
            }
        )
    return in_maps


def gather_output(results):
    out = np.empty((B * T, C), np.float32)
    for c in range(N_CORES):
        out[c * T_OWN : (c + 1) * T_OWN] = results[c]["outT"].T
    return out.reshape(B, T, C)


_CACHED = {}


def kernel(x, w_attn, w_proj):
    if "nc" not in _CACHED:
        _CACHED["nc"] = build_nc()
    in_maps = make_in_maps(x, w_attn, w_proj)
    res = run_bass_kernel_spmd(_CACHED["nc"], in_maps, list(range(N_CORES)))
    return gather_output(res.results)


if __name__ == "__main__":
    rng = np.random.default_rng(0)
    x = rng.standard_normal((B, T, C)).astype(np.float32)
    wa = (rng.standard_normal((C, 3 * C)) / np.sqrt(C)).astype(np.float32)
    wpj = (rng.standard_normal((C, C)) / np.sqrt(C)).astype(np.float32)
    out = kernel(x, wa, wpj)
    print("out", out.shape, out.dtype, np.abs(out).max())



# revision 13
# speedup vs baseline: 1.0461x; 1.0461x over previous
"""Trainium2 Bass kernel for causal local-window self-attention.

Model (matches the PyTorch/JAX reference):
    qkv = x @ w_attn;  q,k,v = split(qkv)
    per head: att = softmax(mask(q k^T / sqrt(hd)));  y = att @ v
    out = y @ w_proj
Shapes (hardcoded): B=2, T=2048, C=1024, H=16, hd=64, window=256.

Sharding: flatten (B,T) -> 4096 tokens, 8 chunks of 512 queries (one per
NeuronCore), each with a 256-token halo of keys/values.  Chunk-boundary
causality (incl. the batch boundary at token 2048) is handled by per-core
additive mask data, so all 8 cores run one identical SPMD program and the
host only reshapes / concatenates.

v2 design notes (vs the earlier f32r build):
  - all matmul operands in bf16 (fp32 PSUM accumulation).  Host-side
    numpy sim of the full quantization chain gives absmax rel err 3.3e-3
    vs the 2e-2 gate.  bf16 stationaries enable FWL (fast weight load).
  - every DMA is contiguous per partition: host pre-lays tensors in the
    exact SBUF layout (no rearrange gathers).
  - softmax denominators come from an appended ones-column of V (row 64
    of the AV matmul output).  Per (head, q-block): DVE reciprocal of
    that row, rank-1 ones-matmul broadcasts it across partitions into
    the same PSUM bank, DVE copy+mul normalizes during evacuation.  No
    DRAM round-trip, no end-of-kernel normalize barrier.
  - program order interleaves projections with per-head attention and
    the qb=0 out-projection with qb=1 attention to keep the PE stream
    dense (HAM stays at K=8/8).
"""

import numpy as np
from ml_dtypes import bfloat16

import concourse.bass as bass
import concourse.mybir as mybir
from concourse.tile import TileContext
from concourse.bass_utils import run_bass_kernel_spmd

F32 = mybir.dt.float32
BF16 = mybir.dt.bfloat16

N_CORES = 8
B, T, C = 2, 2048, 1024
H, HD, W = 16, 64, 256
T_OWN = 512          # queries per core
HALO = 256
T_LOC = T_OWN + HALO  # keys/values per core
NEG = -1e9


# ---------------------------------------------------------------------------
# BIR post-pass: this walrus build only accepts one sync-wait per CTRL-class
# instruction; hoist extra waits onto NoOps inserted just before.
# ---------------------------------------------------------------------------
def _split_excess_waits(nc, max_waits=1):
    for fn in nc.m.functions:
        for blk in fn.blocks:
            insts = blk.instructions
            i = 0
            while i < len(insts):
                inst = insts[i]
                si = inst.sync_info
                if si is not None and si.on_wait and len(si.on_wait) > max_waits:
                    waits = list(si.on_wait)
                    keep = waits[-max_waits:]
                    extra = waits[:-max_waits]
                    nops = []
                    for j in range(0, len(extra), max_waits):
                        nop = mybir.InstNoOp(
                            name=nc.get_next_instruction_name(),
                            sync_info=mybir.SyncInfo(
                                on_wait=extra[j : j + max_waits], on_update=[]
                            ),
                            bass_nofuse=True,
                            engine=inst.engine,
                        )
                        nops.append(nop)
                    inst.sync_info = mybir.SyncInfo(
                        on_wait=keep, on_update=list(si.on_update)
                    )
                    for k, nop in enumerate(nops):
                        insts.insert(i + k, nop)
                        nc.register_instruction(nop)
                    i += len(nops)
                i += 1
    return nc


# ---------------------------------------------------------------------------
# Device program (identical on all 8 cores)
# ---------------------------------------------------------------------------
def build_nc(debug=False, reps=None):
    nc = bass.Bass()

    xT = nc.dram_tensor("xT", [128, 8, T_LOC], BF16, kind="ExternalInput")
    wq = nc.dram_tensor("wq", [128, 8, 8, 128], BF16, kind="ExternalInput")
    wk = nc.dram_tensor("wk", [128, 8, 8, 128], BF16, kind="ExternalInput")
    wv = nc.dram_tensor("wv", [128, 2, 8, 512], BF16, kind="ExternalInput")
    wp = nc.dram_tensor("wp", [128, 8, 8, 128], BF16, kind="ExternalInput")
    maskb = nc.dram_tensor("maskb", [128, 2, 4, 256], F32, kind="ExternalInput")
    outT = nc.dram_tensor("outT", [128, 8, T_OWN], F32, kind="ExternalOutput")
    if debug:
        dbg_q = nc.dram_tensor("dbg_q", [128, 8, T_OWN], F32, kind="ExternalOutput")
        dbg_k = nc.dram_tensor("dbg_k", [128, 8, T_LOC], F32, kind="ExternalOutput")
        dbg_v = nc.dram_tensor("dbg_v", [128, 6, 16, 65], F32, kind="ExternalOutput")
        dbg_p = nc.dram_tensor("dbg_p", [128, 4, 256], F32, kind="ExternalOutput")
        dbg_y = nc.dram_tensor("dbg_y", [128, 8, T_OWN], F32, kind="ExternalOutput")

    with TileContext(nc) as tc:
        with (
            tc.tile_pool(name="res", bufs=1) as res,
            tc.tile_pool(name="pt", bufs=3) as ptpool,
            tc.tile_pool(name="rc", bufs=3) as rcpool,
            tc.tile_pool(name="ot", bufs=2) as otpool,
            tc.tile_pool(name="dbgp", bufs=1) as dbgp,
            tc.tile_pool(name="psq", bufs=2, space="PSUM") as psq,
            tc.tile_pool(name="pss", bufs=3, space="PSUM") as pss_pool,
            tc.tile_pool(name="psy", bufs=3, space="PSUM") as psy_pool,
        ):
          for _rep in range(reps or 1):
            # ---- resident inputs (few big contiguous DMAs, need order) ---
            xts = res.tile([128, 8, T_LOC], BF16, tag="xts")
            nc.sync.dma_start(out=xts[:], in_=xT[:])
            wq_sb = res.tile([128, 8, 8, 128], BF16, tag="wq")
            wk_sb = res.tile([128, 8, 8, 128], BF16, tag="wk")
            wv_sb = res.tile([128, 2, 8, 512], BF16, tag="wv")
            wp_sb = res.tile([128, 8, 8, 128], BF16, tag="wp")
            mk = res.tile([128, 2, 4, 256], F32, tag="mk")
            nc.sync.dma_start(out=wq_sb[:, 0:4], in_=wq[:, 0:4])
            nc.sync.dma_start(out=wk_sb[:, 0:4], in_=wk[:, 0:4])
            nc.sync.dma_start(out=wv_sb[:, 0], in_=wv[:, 0])
            nc.sync.dma_start(out=mk[:], in_=maskb[:])
            nc.sync.dma_start(out=wq_sb[:, 4:8], in_=wq[:, 4:8])
            nc.sync.dma_start(out=wk_sb[:, 4:8], in_=wk[:, 4:8])
            nc.sync.dma_start(out=wv_sb[:, 1], in_=wv[:, 1])
            nc.sync.dma_start(out=wp_sb[:], in_=wp[:])

            ones_sb = res.tile([128, 1], BF16, tag="ones")
            nc.vector.memset(ones_sb[:], 1.0)
            ones_row = res.tile([1, 128], BF16, tag="ones_row")
            nc.vector.memset(ones_row[:], 1.0)

            qTs = res.tile([128, 8, T_OWN], BF16, tag="qTs")
            kTs = res.tile([128, 8, T_LOC], BF16, tag="kTs")
            # V_aug: [part(keys%128), kc, head, 65]; col 64 of each head is 1.0
            vaug = res.tile([128, 6, 16, 65], BF16, tag="vaug")
            yTs = res.tile([128, 8, T_OWN], BF16, tag="yTs")
            # ones column first: no producer deps, so early attention units
            # aren't blocked behind late v-projection writes to this tile
            for kc in range(6):
                nc.vector.tensor_copy(
                    out=vaug[:, kc, :, 64:65],
                    in_=ones_sb[:, None, :].to_broadcast((128, 16, 1)),
                )

            # ---- projection emitters -----------------------------------
            def q_proj(oc):
                ps = psq.tile([128, 512], F32, tag="ps_qkv")
                for ic in range(8):
                    nc.tensor.matmul(
                        ps[:], wq_sb[:, oc, ic], xts[:, ic, HALO:],
                        start=(ic == 0), stop=(ic == 7),
                    )
                nc.scalar.copy(out=qTs[:, oc], in_=ps[:])

            def k_proj(oc):
                for hf in range(2):
                    ps = psq.tile([128, 512], F32, tag="ps_qkv")
                    for ic in range(8):
                        nc.tensor.matmul(
                            ps[:, :384], wk_sb[:, oc, ic],
                            xts[:, ic, hf * 384 : (hf + 1) * 384],
                            start=(ic == 0), stop=(ic == 7),
                        )
                    nc.scalar.copy(
                        out=kTs[:, oc, hf * 384 : (hf + 1) * 384], in_=ps[:, :384]
                    )

            def v_proj(hf):
                for kc in range(6):
                    ps = psq.tile([128, 512], F32, tag="ps_qkv")
                    for ic in range(8):
                        nc.tensor.matmul(
                            ps[:], xts[:, ic, kc * 128 : (kc + 1) * 128],
                            wv_sb[:, hf, ic],
                            start=(ic == 0), stop=(ic == 7),
                        )
                    # ps: [128 tokens, 512 vfeat] -> vaug[:, kc, h, 0:64]
                    nc.scalar.copy(
                        out=vaug[:, kc, hf * 8 : (hf + 1) * 8, 0:64],
                        in_=ps[:].rearrange("p (h d) -> p h d", d=64),
                    )

            # ---- attention unit: one (head, 256-query block) -------------
            def attn(h, qb):
                pb = (h % 2) * 64
                oc = h // 2
                ptile = ptpool.tile([128, 4, 256], BF16, tag="pt")
                for jp in range(2):  # chunk pairs (j=2*jp, 2*jp+1)
                    ps = pss_pool.tile([128, 512], F32, tag="ps_s")
                    for j in (2 * jp, 2 * jp + 1):
                        kc = qb * 2 + j  # key chunk in local coords
                        nc.tensor.matmul(
                            ps[:, (j % 2) * 256 : (j % 2) * 256 + 256],
                            kTs[pb : pb + 64, oc, kc * 128 : (kc + 1) * 128],
                            qTs[pb : pb + 64, oc, qb * 256 : (qb + 1) * 256],
                            start=True, stop=True,
                        )
                    nc.vector.tensor_add(
                        out=ps[:], in0=ps[:],
                        in1=mk[:, qb, 2 * jp : 2 * jp + 2].rearrange("p a t -> p (a t)"),
                    )
                    nc.scalar.activation(
                        out=ptile[:, 2 * jp : 2 * jp + 2].rearrange("p a t -> p (a t)"),
                        in_=ps[:], func=mybir.ActivationFunctionType.Exp,
                    )
                if debug and h == 0 and qb == 0:
                    ptf = dbgp.tile([128, 4, 256], F32, tag="dbg")
                    nc.vector.tensor_copy(out=ptf[:], in_=ptile[:])
                    nc.sync.dma_start(out=dbg_p[:], in_=ptf[:])
                # AV (+ ones row -> denominators) and rank-1 recip broadcast
                ya = psy_pool.tile([128, 512], F32, tag="ps_y")
                for j in range(4):
                    kc = qb * 2 + j
                    nc.tensor.matmul(
                        ya[:65, 0:256], vaug[:, kc, h], ptile[:, j],
                        start=(j == 0), stop=(j == 3),
                    )
                recip = rcpool.tile([1, 256], BF16, tag="recip")
                # bf16 recip adds ~2e-3 rel err on the normalization; the
                # full-pipeline numpy sim lands at 3.3e-3 vs the 2e-2 gate.
                # (reciprocal_approx_fast would be 5x cheaper but its custom
                # DVE uop fails this walrus build's visitInstISA check)
                with nc.allow_low_precision(reason="bf16 softmax recip, sim 3.3e-3"):
                    nc.vector.reciprocal(out=recip[:], in_=ya[64:65, 0:256])
                nc.tensor.matmul(
                    ya[:, 256:512], ones_row[:], recip[:], start=True, stop=True
                )
                # evacuate + normalize: yT = ya[0:64] * recip_bcast
                nc.vector.tensor_copy(
                    out=yTs[pb : pb + 64, oc, qb * 256 : (qb + 1) * 256],
                    in_=ya[0:64, 0:256],
                )
                nc.vector.tensor_mul(
                    out=yTs[pb : pb + 64, oc, qb * 256 : (qb + 1) * 256],
                    in0=yTs[pb : pb + 64, oc, qb * 256 : (qb + 1) * 256],
                    in1=ya[pb : pb + 64, 256:512],
                )

            # ---- out-projection for one (oc, 256-query block) ------------
            def out_proj(oc, qb):
                ps = psq.tile([128, 512], F32, tag="ps_qkv")
                for ic in range(8):
                    nc.tensor.matmul(
                        ps[:, 0:256], wp_sb[:, oc, ic],
                        yTs[:, ic, qb * 256 : (qb + 1) * 256],
                        start=(ic == 0), stop=(ic == 7),
                    )
                ot = otpool.tile([128, 256], F32, tag="ot")
                nc.vector.tensor_copy(out=ot[:], in_=ps[:, 0:256])
                nc.sync.dma_start(
                    out=outT[:, oc, qb * 256 : (qb + 1) * 256], in_=ot[:]
                )

            # ---- PE program order (kept dense) ---------------------------
            q_proj(0); k_proj(0); v_proj(0)
            q_proj(1); k_proj(1)
            attn(0, 0); attn(1, 0)
            q_proj(2); k_proj(2)
            attn(2, 0); attn(3, 0)
            q_proj(3); k_proj(3)
            attn(4, 0); attn(5, 0)
            v_proj(1)
            q_proj(4); k_proj(4)
            attn(6, 0); attn(7, 0)
            q_proj(5); k_proj(5)
            attn(8, 0); attn(9, 0)
            q_proj(6); k_proj(6)
            attn(10, 0); attn(11, 0)
            q_proj(7); k_proj(7)
            attn(12, 0); attn(13, 0)
            attn(14, 0); attn(15, 0)
            # qb=1 attention interleaved with qb=0 out-projection
            for t in range(8):
                attn(2 * t, 1)
                attn(2 * t + 1, 1)
                out_proj(t, 0)
            for oc in range(8):
                out_proj(oc, 1)

            if debug:
                for src, dst, shp in (
                    (qTs, dbg_q, [128, 8, T_OWN]),
                    (kTs, dbg_k, [128, 8, T_LOC]),
                    (vaug, dbg_v, [128, 6, 16, 65]),
                    (yTs, dbg_y, [128, 8, T_OWN]),
                ):
                    dtile = dbgp.tile(shp, F32, tag="dbg")
                    nc.vector.tensor_copy(out=dtile[:], in_=src[:])
                    nc.sync.dma_start(out=dst[:], in_=dtile[:])

    _split_excess_waits(nc)
    return nc


# ---------------------------------------------------------------------------
# Host-side sharding / unsharding
# ---------------------------------------------------------------------------
def _build_masks():
    """mask[qb, j, r, col]: 0 if query col of q-block qb may attend key
    qb*256+j*128+r (local coords), else NEG.  Variant 0: interior chunk;
    variant 1: first chunk of a batch (halo keys are invalid)."""
    r = np.arange(128)[:, None]
    col = np.arange(256)[None, :]
    masks = []
    for chunk0 in (False, True):
        m = np.full((2, 4, 128, 256), NEG, np.float32)
        for qb in range(2):
            for j in range(4):
                d = j * 128 + r  # key pos relative to q-block start
                valid = (col >= d - 256) & (col < d)
                if chunk0:
                    valid &= (d + qb * 256) >= 256
                m[qb, j][valid] = 0.0
        masks.append(m.transpose(2, 0, 1, 3).copy())  # -> [p, qb, j, col]
    return masks  # [interior, chunk0]


def make_in_maps(x, w_attn, w_proj):
    xf = np.ascontiguousarray(x.reshape(B * T, C)).astype(np.float32)
    wq = (w_attn[:, :C] * np.float32(1.0 / np.sqrt(HD))).astype(np.float32)
    wk = np.ascontiguousarray(w_attn[:, C : 2 * C])
    wv = np.ascontiguousarray(w_attn[:, 2 * C :])
    wp = np.ascontiguousarray(w_proj).astype(np.float32)

    def wlayout(w):  # [C, C] -> [p, oc, ic, m]
        return np.ascontiguousarray(
            w.reshape(8, 128, 8, 128).transpose(1, 2, 0, 3)
        ).astype(bfloat16)

    wq_l = wlayout(wq)
    wk_l = wlayout(wk)
    wp_l = wlayout(wp)
    # wv: [p, hf, ic, m] with m over 512 vfeats
    wv_l = np.ascontiguousarray(
        wv.reshape(8, 128, 2, 512).transpose(1, 2, 0, 3)
    ).astype(bfloat16)
    mask_int, mask_c0 = _build_masks()

    in_maps = []
    for c in range(N_CORES):
        start = c * T_OWN
        xpad = np.zeros((T_LOC, C), np.float32)
        if c % 4 == 0:
            xpad[HALO:] = xf[start : start + T_OWN]
            m = mask_c0
        else:
            xpad[:] = xf[start - HALO : start + T_OWN]
            m = mask_int
        xT_l = np.ascontiguousarray(
            xpad.T.reshape(8, 128, T_LOC).transpose(1, 0, 2)
        ).astype(bfloat16)
        in_maps.append(
            {
                "xT": xT_l,
                "wq": wq_l,
                "wk": wk_l,
                "wv": wv_l,
                "wp": wp_l,
                "maskb": m,
            }
        )
    return in_maps


def gather_output(results):
    out = np.empty((B * T, C), np.float32)
    for c in range(N_CORES):
        # outT: [p, oc, t] -> [t, oc*128+p]
        ot = results[c]["outT"]
        out[c * T_OWN : (c + 1) * T_OWN] = (
            ot.transpose(1, 0, 2).reshape(C, T_OWN).T
        )
    return out.reshape(B, T, C)


_CACHED = {}


def kernel(x, w_attn, w_proj):
    if "nc" not in _CACHED:
        _CACHED["nc"] = build_nc()
    in_maps = make_in_maps(x, w_attn, w_proj)
    res = run_bass_kernel_spmd(_CACHED["nc"], in_maps, list(range(N_CORES)))
    return gather_output(res.results)


if __name__ == "__main__":
    rng = np.random.default_rng(0)
    x = rng.standard_normal((B, T, C)).astype(np.float32)
    wa = (rng.standard_normal((C, 3 * C)) / np.sqrt(C)).astype(np.float32)
    wpj = (rng.standard_normal((C, C)) / np.sqrt(C)).astype(np.float32)
    out = kernel(x, wa, wpj)
    print("out", out.shape, out.dtype, np.abs(out).max())
